# revision 7
# baseline (speedup 1.0000x reference)
import os
import sys
from contextlib import ExitStack

import numpy as np

for _p in ("/opt/trn_rl_repo",):
    if os.path.isdir(_p) and _p not in sys.path:
        sys.path.insert(0, _p)

# Problem (nn_PosDecoder): out[n,l] = sum_c src[n,l,:128] . (table[1+c]*sqrt(128))
#   = src[n,l,:128] . colsum  where colsum = sqrt(128) * sum(table[1:], axis=0).
# Shard table rows across 8 cores; each core computes a partial colsum and a
# partial (N,L) output row; host sums the 8 partial rows.
#
# Host relayout: core k's 12500 rows -> 97 blocks of (128,128) -> chunks that
# are f-major inside (col = f*B + b), so the on-device fold is a contiguous
# innermost reduce. Colsum is permutation/associativity-invariant, so any
# grouping of rows is valid.
N, L, M = 16, 100, 256
F = 128
N_LOC = 100001
N_CORES = 8
R = (N_LOC - 1) // N_CORES  # 12500 table rows per core
TOK = N * L  # 1600
NBLK = R // 128  # 97 full 128-row blocks
TAIL = R - NBLK * 128  # 84
SCALE = float(np.sqrt(F))

# chunk plan: 11 wide (8 blocks), 2 narrow (4 blocks), 1 single
CHUNK_PLAN = [(i * 8, 8) for i in range(11)] + [(88, 4), (92, 4), (96, 1)]
OFFS = np.cumsum([0] + [nb * F for _, nb in CHUNK_PLAN]).tolist()
WIDE_W = 8 * F   # 1024
NARROW_W = 4 * F  # 512

_BUILT = None


def _build():
    import concourse.bass as bass
    import concourse.tile as tile
    from concourse import bacc, mybir

    nc = bacc.Bacc("TRN2", target_bir_lowering=False, debug=False,
                   num_devices=N_CORES)
    f32 = mybir.dt.float32
    tabX = nc.dram_tensor("tabX", (128, NBLK * F), f32,
                          kind="ExternalInput").ap()
    tail84 = nc.dram_tensor("tail84", (TAIL, F), f32,
                            kind="ExternalInput").ap()
    srcT = nc.dram_tensor("srcT", (F, TOK), f32, kind="ExternalInput").ap()
    out = nc.dram_tensor("out", (1, TOK), f32, kind="ExternalOutput").ap()

    def seg(i):
        return tabX[:, OFFS[i]:OFFS[i + 1]]

    with tile.TileContext(nc) as tc, ExitStack() as ctx:
        sb = ctx.enter_context(tc.tile_pool(name="sb", bufs=1))
        schunks = ctx.enter_context(tc.tile_pool(name="schunks", bufs=3))
        achunks = ctx.enter_context(tc.tile_pool(name="achunks", bufs=3))
        parts = ctx.enter_context(tc.tile_pool(name="parts", bufs=2))
        psum1 = ctx.enter_context(
            tc.tile_pool(name="psum1", bufs=1, space=bass.MemorySpace.PSUM))
        psumv = ctx.enter_context(
            tc.tile_pool(name="psumv", bufs=4, space=bass.MemorySpace.PSUM))

        ones = sb.tile([128, 1], f32)
        nc.gpsimd.memset(ones[:], SCALE)  # folds the sqrt(F) scale into colsum
        srcT_sb = sb.tile([128, TOK], f32)
        out_sb = sb.tile([1, TOK], f32)
        acc1 = sb.tile([128, WIDE_W], f32)
        acc2 = sb.tile([128, NARROW_W], f32)
        g0 = sb.tile([128, F], f32)
        tailt = sb.tile([TAIL, F], f32)

        # --- sync queue: W0 (slivered, direct into acc1), W2,W4,W6,W8,W10,
        #     N0 (chunk 11), srcT cols 1024:1536, out at the end.
        for i in range(8):
            nc.sync.dma_start(acc1[16 * i:16 * (i + 1), :],
                              seg(0)[16 * i:16 * (i + 1), :])
        s_tiles = []
        for ci in (2, 4, 6, 8, 10):
            ch = schunks.tile([128, WIDE_W], f32)
            nc.sync.dma_start(ch[:], seg(ci))
            s_tiles.append(ch)
        n0 = schunks.tile([128, NARROW_W], f32)
        nc.sync.dma_start(n0[:], seg(11))
        nc.sync.dma_start(srcT_sb[:, 1024:1536], srcT[:, 1024:1536])

        # --- act queue: W1 (slivered), W3,W5,W7,W9, g0 (chunk 13), tail rows,
        #     N1 (chunk 12, direct into acc2), srcT cols 0:512, 512:1024,
        #     1536:1600.
        w1 = achunks.tile([128, WIDE_W], f32)
        for i in range(8):
            nc.scalar.dma_start(w1[16 * i:16 * (i + 1), :],
                                seg(1)[16 * i:16 * (i + 1), :])
        a_tiles = [w1]
        for ci in (3, 5, 7, 9):
            ch = achunks.tile([128, WIDE_W], f32)
            nc.scalar.dma_start(ch[:], seg(ci))
            a_tiles.append(ch)
        nc.scalar.dma_start(g0[:], seg(13))
        nc.scalar.dma_start(tailt[:], tail84[:, :])
        nc.scalar.dma_start(acc2[:], seg(12))
        nc.scalar.dma_start(srcT_sb[:, 0:512], srcT[:, 0:512])
        nc.scalar.dma_start(srcT_sb[:, 512:1024], srcT[:, 512:1024])
        nc.scalar.dma_start(srcT_sb[:, 1536:1600], srcT[:, 1536:1600])

        # --- DVE: acc1 += W1..W10 in arrival order, contiguous fold, then
        #     acc2 += N0 and its fold.
        order = []
        for i in range(5):
            order.append(a_tiles[i])
            order.append(s_tiles[i])
        for ch in order:
            nc.vector.tensor_add(acc1[:], acc1[:], ch[:])
        part1 = parts.tile([128, F], f32)
        nc.vector.tensor_reduce(
            part1[:], acc1.rearrange("p (f b) -> p f b", f=F),
            axis=mybir.AxisListType.X, op=mybir.AluOpType.add)
        nc.vector.tensor_add(acc2[:], acc2[:], n0[:])
        part2 = parts.tile([128, F], f32)
        nc.vector.tensor_reduce(
            part2[:], acc2.rearrange("p (f b) -> p f b", f=F),
            axis=mybir.AxisListType.X, op=mybir.AluOpType.add)

        # --- PE: one PSUM accumulation group -> colsum (128,1)
        cps = psum1.tile([128, 1], f32)
        nc.tensor.matmul(cps[:], g0[:], ones[:], start=True, stop=False)
        nc.tensor.matmul(cps[:], tailt[:], ones[:TAIL, :], start=False,
                         stop=False)
        nc.tensor.matmul(cps[:], part1[:], ones[:], start=False, stop=False)
        nc.tensor.matmul(cps[:], part2[:], ones[:], start=False, stop=True)
        colsum = sb.tile([128, 1], f32)
        nc.vector.tensor_copy(colsum[:], cps[:])

        # --- out_row = colsum^T @ srcT -> (1, 1600)
        for j in range(0, TOK, 512):
            w = min(512, TOK - j)
            pv = psumv.tile([1, 512], f32)
            nc.tensor.matmul(pv[:1, :w], colsum[:], srcT_sb[:, j:j + w],
                             start=True, stop=True)
            nc.vector.tensor_copy(out_sb[:, j:j + w], pv[:1, :w])
        nc.sync.dma_start(out[:], out_sb[:])

    nc.compile()
    return nc


def make_in_maps(src, lookup_table):
    src_f = np.asarray(src, dtype=np.float32).reshape(TOK, M)[:, :F]
    srcT_np = np.ascontiguousarray(src_f.T)  # (128, 1600)
    tab = np.asarray(lookup_table, dtype=np.float32)
    in_maps = []
    for k in range(N_CORES):
        sl = tab[1 + k * R:1 + (k + 1) * R, :]
        blocks = sl[:NBLK * 128].reshape(128, NBLK, F)  # [p, t, f]
        segs = []
        for b0, nb in CHUNK_PLAN:
            sub = blocks[:, b0:b0 + nb, :]  # [p, b, f] -> f-major [p, f, b]
            segs.append(sub.transpose(0, 2, 1).reshape(128, F * nb))
        tabX = np.ascontiguousarray(np.concatenate(segs, axis=1))
        tail_np = np.ascontiguousarray(sl[NBLK * 128:])
        in_maps.append({"tabX": tabX, "tail84": tail_np, "srcT": srcT_np})
    return in_maps


def kernel(src=None, ds=None, lookup_table=None, **_):
    global _BUILT
    if _BUILT is None:
        _BUILT = _build()
    from concourse import bass_utils

    in_maps = make_in_maps(src, lookup_table)
    res = bass_utils.run_bass_kernel_spmd(_BUILT, in_maps,
                                          core_ids=list(range(N_CORES)))
    parts = [next(iter(r.values())).reshape(-1) for r in res.results]
    total = np.sum(np.stack(parts, 0), axis=0, dtype=np.float64)
    return total.astype(np.float32).reshape(N, L)


# revision 8
# speedup vs baseline: 1.1472x; 1.1472x over previous
import os
import sys
from contextlib import ExitStack

import numpy as np

for _p in ("/opt/trn_rl_repo",):
    if os.path.isdir(_p) and _p not in sys.path:
        sys.path.insert(0, _p)

# Problem (nn_PosDecoder): out[n,l] = sum_c src[n,l,:128] . (table[1+c]*sqrt(128))
#   = src[n,l,:128] . colsum  where colsum = sqrt(128) * sum(table[1:], axis=0).
# Shard table rows across 8 cores; each core computes a partial colsum and a
# partial (N,L) output row; host sums the 8 partial rows.
#
# Host relayout: core k's 12500 rows -> 97 blocks of (128,128) -> 12 wide
# chunks of 8 blocks, f-major inside (col = f*8 + b) so the on-device fold is
# a contiguous innermost reduce, plus one single block (g0, streamed last,
# fed straight to PE) and an 84-row tail. Colsum is permutation/associativity
# invariant, so any grouping of rows is valid.
N, L, M = 16, 100, 256
F = 128
N_LOC = 100001
N_CORES = 8
R = (N_LOC - 1) // N_CORES  # 12500 table rows per core
TOK = N * L  # 1600
NBLK = R // 128  # 97 full 128-row blocks
TAIL = R - NBLK * 128  # 84
SCALE = float(np.sqrt(F))

CHUNK_PLAN = [(i * 8, 8) for i in range(12)] + [(96, 1)]
OFFS = np.cumsum([0] + [nb * F for _, nb in CHUNK_PLAN]).tolist()
WIDE_W = 8 * F  # 1024

_BUILT = None


def _build():
    import concourse.bass as bass
    import concourse.tile as tile
    from concourse import bacc, mybir

    nc = bacc.Bacc("TRN2", target_bir_lowering=False, debug=False,
                   num_devices=N_CORES)
    f32 = mybir.dt.float32
    tabX = nc.dram_tensor("tabX", (128, NBLK * F), f32,
                          kind="ExternalInput").ap()
    tail84 = nc.dram_tensor("tail84", (TAIL, F), f32,
                            kind="ExternalInput").ap()
    srcT = nc.dram_tensor("srcT", (F, TOK), f32, kind="ExternalInput").ap()
    out = nc.dram_tensor("out", (1, TOK), f32, kind="ExternalOutput").ap()

    def seg(i):
        return tabX[:, OFFS[i]:OFFS[i + 1]]

    with tile.TileContext(nc) as tc, ExitStack() as ctx:
        sb = ctx.enter_context(tc.tile_pool(name="sb", bufs=1))
        schunks = ctx.enter_context(tc.tile_pool(name="schunks", bufs=4))
        achunks = ctx.enter_context(tc.tile_pool(name="achunks", bufs=4))
        psum1 = ctx.enter_context(
            tc.tile_pool(name="psum1", bufs=1, space=bass.MemorySpace.PSUM))
        psumv = ctx.enter_context(
            tc.tile_pool(name="psumv", bufs=4, space=bass.MemorySpace.PSUM))

        ones = sb.tile([128, 1], f32)
        nc.gpsimd.memset(ones[:], SCALE)  # folds the sqrt(F) scale into colsum
        srcT_sb = sb.tile([128, TOK], f32)
        out_sb = sb.tile([1, TOK], f32)
        acc1 = sb.tile([128, WIDE_W], f32)
        g0 = sb.tile([128, F], f32)
        tailt = sb.tile([TAIL, F], f32)

        # --- sync queue: W0 direct into acc1, W2..W10, srcT halves, out at end
        nc.sync.dma_start(acc1[:], seg(0))
        s_tiles = []
        for ci in (2, 4, 6, 8, 10):
            ch = schunks.tile([128, WIDE_W], f32)
            nc.sync.dma_start(ch[:], seg(ci))
            s_tiles.append(ch)
        nc.sync.dma_start(srcT_sb[:, 0:512], srcT[:, 0:512])
        nc.sync.dma_start(srcT_sb[:, 512:1024], srcT[:, 512:1024])

        # --- act queue: W1, tail (early, feeds the early tail matmul),
        #     W3..W11, srcT tail cols, g0 dead last.
        w1 = achunks.tile([128, WIDE_W], f32)
        nc.scalar.dma_start(w1[:], seg(1))
        nc.scalar.dma_start(tailt[:], tail84[:, :])
        a_tiles = [w1]
        for ci in (3, 5, 7, 9, 11):
            ch = achunks.tile([128, WIDE_W], f32)
            nc.scalar.dma_start(ch[:], seg(ci))
            a_tiles.append(ch)
        nc.scalar.dma_start(srcT_sb[:, 1024:1600], srcT[:, 1024:1600])
        nc.scalar.dma_start(g0[:], seg(12))

        # --- DVE: acc1 += W1..W11 in arrival order, then contiguous fold.
        order = []
        for i in range(6):
            order.append(a_tiles[i])
            if i < 5:
                order.append(s_tiles[i])
        for ch in order:
            nc.vector.tensor_add(acc1[:], acc1[:], ch[:])
        part1 = sb.tile([128, F], f32)
        nc.vector.tensor_reduce(
            part1[:], acc1.rearrange("p (f b) -> p f b", f=F),
            axis=mybir.AxisListType.X, op=mybir.AluOpType.add)

        # --- PE: one PSUM group -> colsum (128,1); g0 closes it (last DMA).
        cps = psum1.tile([128, 1], f32)
        nc.tensor.matmul(cps[:], tailt[:], ones[:TAIL, :], start=True,
                         stop=False)
        nc.tensor.matmul(cps[:], part1[:], ones[:], start=False, stop=False)
        nc.tensor.matmul(cps[:], g0[:], ones[:], start=False, stop=True)
        colsum = sb.tile([128, 1], f32)
        nc.vector.tensor_copy(colsum[:], cps[:])

        # --- out_row = colsum^T @ srcT -> (1, 1600); per-slice copy + DMA out
        for j in range(0, TOK, 512):
            w = min(512, TOK - j)
            pv = psumv.tile([1, 512], f32)
            nc.tensor.matmul(pv[:1, :w], colsum[:], srcT_sb[:, j:j + w],
                             start=True, stop=True)
            nc.vector.tensor_copy(out_sb[:, j:j + w], pv[:1, :w])
            nc.sync.dma_start(out[:, j:j + w], out_sb[:, j:j + w])

    nc.compile()
    return nc


def make_in_maps(src, lookup_table):
    src_f = np.asarray(src, dtype=np.float32).reshape(TOK, M)[:, :F]
    srcT_np = np.ascontiguousarray(src_f.T)  # (128, 1600)
    tab = np.asarray(lookup_table, dtype=np.float32)
    in_maps = []
    for k in range(N_CORES):
        sl = tab[1 + k * R:1 + (k + 1) * R, :]
        blocks = sl[:NBLK * 128].reshape(128, NBLK, F)  # [p, t, f]
        segs = []
        for b0, nb in CHUNK_PLAN:
            sub = blocks[:, b0:b0 + nb, :]  # [p, b, f] -> f-major [p, f, b]
            segs.append(sub.transpose(0, 2, 1).reshape(128, F * nb))
        tabX = np.ascontiguousarray(np.concatenate(segs, axis=1))
        tail_np = np.ascontiguousarray(sl[NBLK * 128:])
        in_maps.append({"tabX": tabX, "tail84": tail_np, "srcT": srcT_np})
    return in_maps


def kernel(src=None, ds=None, lookup_table=None, **_):
    global _BUILT
    if _BUILT is None:
        _BUILT = _build()
    from concourse import bass_utils

    in_maps = make_in_maps(src, lookup_table)
    res = bass_utils.run_bass_kernel_spmd(_BUILT, in_maps,
                                          core_ids=list(range(N_CORES)))
    parts = [next(iter(r.values())).reshape(-1) for r in res.results]
    total = np.sum(np.stack(parts, 0), axis=0, dtype=np.float64)
    return total.astype(np.float32).reshape(N, L)
